# revision 67
# baseline (speedup 1.0000x reference)
"""Allegro-style GNN energy kernel on 8 Trainium2 NeuronCores (Bass/Tile), v3.

Edge-parallel across 8 cores (100k edges each). Per core:
 - pos gathered from a 64B-row pair table (parity select), species data
   gathered from a [10000, 384] bf16 (z_r, z_c)-pair table with
   transpose-mode dma_gather, landing feature-on-partition:
     chunk0 = D0 (A_pair@w1a0 + c1_0, 64) | c1_1 (64)
     chunk1 = CA (A_pair@Cm, 128)
     chunk2 = c1_2 (64) | zeros
 - layer-0 is folded: ef0 = A + Wgeo@bT is never materialized; its two
   matmul consumers absorb it (w1a0 folded into combo0/D0, CmatX@ef0
   becomes CmGeo@bT + CA-inject).
 - geometry computed edge-on-partition in engine-phased order (all Rsqrt,
   then all Exp, then all Sin per group) to minimize ACT table loads.
 - stationary weights replicated at base partition 64 (I64b/w3C64/tp2g64)
   so the tp/j=1 matmul operands slice hs/h2s/S directly (no DVE copies).
 - LayerNorm: xc copied PSUM->SBUF once (DVE reads at most one PSUM
   operand per op); per-pair rstd Rsqrt calls dependency-clustered per
   (group, layer) to avoid act-table thrash; ef = xw * rstd in-place.
 - per-edge energies accumulate in one PSUM bank; per-core partial sums
   are added on the host (the all-reduce of the sharding hint). atomic_e
   and E*hb3 are folded into a host-side bias constant.
"""
import math
import os
import time
import numpy as np
import ml_dtypes

import concourse.bass as bass
import concourse.bacc as bacc
import concourse.tile as tile
from concourse import mybir
import concourse.bass_isa as bisa
from concourse.bass_utils import run_bass_kernel_spmd

NN = 50000
H = 64
DD = 128
NSH = 9
NB = 8
NL = 3
CUTOFF = 5.0
LN_EPS = 1e-5
N_CORES = 8

TILE = 512
SUPER = 2048
CH = SUPER // 128      # 16 chunks of 128 edges per supertile
GSUP = 4

F32 = mybir.dt.float32
BF16 = mybir.dt.bfloat16
I16 = mybir.dt.int16
U8 = mybir.dt.uint8
AF = mybir.ActivationFunctionType
ALU = mybir.AluOpType

CENTS = np.cos((2 * np.arange(1, NB + 1) - 1) * math.pi / (2 * NB)) * CUTOFF
C1 = 0.4886025119029199
C2 = 1.0925484305920792


def _sh_mix_matrix():
    """S such that sh = S @ ab9, ab9 = [xx,yy,zz,xy,yz,xz,y,z,x] (|u|=1)."""
    S = np.zeros((9, 9), np.float32)
    S[0, 0] = S[0, 1] = S[0, 2] = 0.28209479177387814
    S[1, 6] = -C1
    S[2, 7] = C1
    S[3, 8] = -C1
    S[4, 3] = C2
    S[5, 4] = -C2
    S[6, 0] = S[6, 1] = -0.31539156525252
    S[6, 2] = 0.94617469575756
    S[7, 5] = -C2
    S[8, 0] = 0.5462742152960396
    S[8, 1] = -0.5462742152960396
    return S


def _wrap_idx(idx16, per_call):
    blocks = []
    for s in range(0, len(idx16), per_call):
        w = idx16[s:s + per_call].reshape(-1, 16).T
        blocks.append(np.tile(w, (8, 1)))
    return np.concatenate(blocks, axis=1).copy()


def _act_raw(nc, out, in_, func, bias=0.0, scale=1.0):
    """InstActivation without the bass accuracy guards (Rsqrt path)."""
    import concourse.bass as _b
    eng = nc.scalar
    inputs = [eng.lower_ap(in_)]
    for arg in (bias, scale, 0.0):
        if isinstance(arg, _b.AP) or not isinstance(arg, float):
            inputs.append(eng.lower_ap(arg))
        else:
            inputs.append(mybir.ImmediateValue(dtype=mybir.dt.float32, value=arg))
    return eng.add_instruction(
        mybir.InstActivation(
            name=nc.get_next_instruction_name(),
            func=func, ins=inputs, outs=[eng.lower_ap(out)]))


def _build(nc, EC_PAD, NAT_PAD, nz):
    NT = EC_PAD // TILE
    NS = EC_PAD // SUPER

    dt = nc.dram_tensor
    d_in = {}

    def din(name, shape, dtype):
        d_in[name] = dt(name, list(shape), dtype, kind="ExternalInput")
        return d_in[name]

    ptab_d = din("ptab", [(NN + 1) // 2, 64], F32)
    spt_d = din("spt", [10000, 384], BF16)
    idxr_d = din("idxr", [128, EC_PAD // 16], I16)
    idxc_d = din("idxc", [128, EC_PAD // 16], I16)
    idxs_d = din("idxs", [128, EC_PAD // 16], I16)
    parr_d = din("parr", [128, EC_PAD // 128, 3], U8)
    parc_d = din("parc", [128, EC_PAD // 128, 3], U8)

    wdefs16 = {
        "I64": [64, 64], "I64b": [128, 64], "ones1": [128, 1],
        "ident": [128, 128], "CmGeo": [17, 128], "CmatX": [128, 128],
        "hw1": [DD, H], "hw2st": [128, H], "hw3cat": [128, 1],
    }
    for l in range(NL):
        wdefs16.update({
            f"combo{l}": [17, 128],
            f"w2{l}": [H, H], f"w3C{l}": [H, DD], f"w3C64_{l}": [128, DD],
            f"tp2g64_{l}": [128, DD],
        })
    for l in range(1, NL):
        wdefs16[f"w1a{l}"] = [DD, H]
    for k, shp in wdefs16.items():
        din(k, shp, BF16)
    wdefs32 = {"ebias": [1, 1], "halfpi": [128, 1], "cents8": [128, 1, NB]}
    for k, shp in wdefs32.items():
        din(k, shp, F32)

    out_d = dt("out", [1, 1], F32, kind="ExternalOutput")

    with tile.TileContext(nc) as tc:
        with (
            tc.tile_pool(name="const", bufs=1) as cp,
            tc.tile_pool(name="land", bufs=2) as lp,
            tc.tile_pool(name="geo", bufs=2) as gp,
            tc.tile_pool(name="feat", bufs=2) as fp,
            tc.tile_pool(name="psum", bufs=1, space=bass.MemorySpace.PSUM) as pp,
        ):
            _uid = [0]

            def T(pool, shape, dtype, tag, bufs=None):
                _uid[0] += 1
                return pool.tile(list(shape), dtype, tag=tag,
                                 name=f"{tag}_{_uid[0]}", bufs=bufs)

            ct = {}
            for k in list(wdefs16) + list(wdefs32):
                dtt = BF16 if k in wdefs16 else F32
                ct[k] = T(cp, d_in[k].shape, dtt, k)
                nc.sync.dma_start(ct[k][:], d_in[k].ap()[:])
            idx_d = {"idxr": idxr_d, "idxc": idxc_d, "idxs": idxs_d}
            par_t = {}
            for nm, d_ in (("parr", parr_d), ("parc", parc_d)):
                par_t[nm] = T(cp, [128, EC_PAD // 128, 3], U8, nm + "_t")
                nc.sync.dma_start(par_t[nm][:], d_.ap()[:])

            acc_sb = T(cp, [1, TILE], F32, "acc_sb")
            nc.vector.memset(acc_sb[:], 0.0)
            NEC_VALID = 800000 // N_CORES - (NT - 1) * TILE

            sgroups = []
            s0 = 0
            while s0 < NS:
                sgroups.append(list(range(s0, min(s0 + GSUP, NS))))
                s0 += GSUP

            from concourse.tile_rust import add_dep_helper as _adh

            def adh(a, b, reason):
                _i1 = getattr(a, "ins", a)
                _i2 = getattr(b, "ins", b)
                try:
                    _adh(_i1, _i2, sync=False, reason=reason)
                except TypeError:
                    _adh(_i1, _i2, False, reason)

            for grp in sgroups:
                ng = len(grp)
                g0 = grp[0] * (SUPER // 16)
                g1 = (grp[-1] + 1) * (SUPER // 16)
                idx_t = {}
                for nm in ("idxr", "idxc", "idxs"):
                    idx_t[nm] = T(lp, [128, g1 - g0], I16, nm + "_g", bufs=2)
                    nc.sync.dma_start(idx_t[nm][:], idx_d[nm].ap()[:, g0:g1])

                # ---------- gathers per supertile ----------
                Ss, landrs, landcs = {}, {}, {}
                for s in grp:
                    land_r = T(lp, [128, CH, 64], F32, "land_r")
                    land_c = T(lp, [128, CH, 64], F32, "land_c")
                    for land, inm in ((land_r, "idxr"), (land_c, "idxc")):
                        for k in range(2):
                            i0 = (s - grp[0]) * (SUPER // 16) + k * 64
                            nc.gpsimd.dma_gather(
                                land[:, k * 8:(k + 1) * 8, :], ptab_d.ap()[:],
                                idx_t[inm][:, i0:i0 + 64], 1024, 1024, 64,
                                queue_num=0)
                    S = T(lp, [128, 4, 3, TILE], BF16, "spec", bufs=GSUP + 1)
                    for k in range(4):
                        i0 = (s - grp[0]) * (SUPER // 16) + k * 32
                        nc.gpsimd.dma_gather(
                            S[:, k, :, :], spt_d.ap()[:],
                            idx_t["idxs"][:, i0:i0 + 32], TILE, TILE, 384,
                            transpose=True, queue_num=1)
                    Ss[s] = S
                    landrs[s], landcs[s] = land_r, land_c

                # ---------- geometry, engine-phased across the group ------
                # d2/db/gt/s1/gx live in group-merged tiles so Rsqrt/Sin/Exp
                # run ONCE per group (one act-table visit each instead of 4)
                GB = GSUP + 1
                bunds, bTs, vts = {}, {}, {}
                d2_g = T(gp, [128, GSUP, CH, 1], F32, "d2", bufs=2)
                for si, s in enumerate(grp):
                    land_r, land_c = landrs[s], landcs[s]
                    pr = T(gp, [128, CH, 3], F32, "pr", bufs=2)
                    nc.vector.tensor_copy(pr[:], land_r[:, :, 0:3])
                    nc.vector.copy_predicated(
                        pr[:], par_t["parr"][:, s * CH:(s + 1) * CH, :],
                        land_r[:, :, 32:35])
                    pc = T(gp, [128, CH, 3], F32, "pc", bufs=2)
                    nc.vector.tensor_copy(pc[:], land_c[:, :, 0:3])
                    nc.vector.copy_predicated(
                        pc[:], par_t["parc"][:, s * CH:(s + 1) * CH, :],
                        land_c[:, :, 32:35])
                    vt = T(gp, [128, CH, 3], F32, "vt", bufs=GB)
                    nc.vector.tensor_tensor(vt[:], pc[:], pr[:], op=ALU.subtract)
                    vts[s] = vt
                    sqt = T(gp, [128, CH, 3], F32, "sqt", bufs=2)
                    nc.vector.tensor_tensor(sqt[:], vt[:], vt[:], op=ALU.mult)
                    nc.vector.tensor_reduce(d2_g[:, si, :, :], sqt[:],
                                            axis=mybir.AxisListType.X, op=ALU.add)
                rinv_g = T(gp, [128, GSUP, CH, 1], F32, "rinv", bufs=2)
                _act_raw(nc, rinv_g[:, 0:ng, :, :], d2_g[:, 0:ng, :, :],
                         AF.Rsqrt, bias=1e-16)
                db_g = T(gp, [128, GSUP, CH, 1], F32, "db", bufs=2)
                gt_g = T(gp, [128, GSUP, CH, NB], F32, "gt", bufs=2)
                for si, s in enumerate(grp):
                    vt = vts[s]
                    nc.vector.tensor_tensor(db_g[:, si, :, :], d2_g[:, si, :, :],
                                            rinv_g[:, si, :, :], op=ALU.mult)
                    nc.vector.tensor_tensor(
                        gt_g[:, si, :, :],
                        db_g[:, si, :, :].broadcast_to([128, CH, NB]),
                        ct["cents8"][:].broadcast_to([128, CH, NB]),
                        op=ALU.subtract)
                    ut = T(gp, [128, CH, 3], F32, "ut", bufs=2)
                    nc.vector.tensor_tensor(
                        ut[:], vt[:],
                        rinv_g[:, si, :, :].broadcast_to([128, CH, 3]),
                        op=ALU.mult)
                    bund = T(gp, [128, CH, 32], BF16, "bund", bufs=3)
                    nc.vector.tensor_tensor(bund[:, :, 0:3], ut[:], ut[:],
                                            op=ALU.mult)
                    nc.vector.tensor_tensor(bund[:, :, 3:5], ut[:, :, 0:2],
                                            ut[:, :, 1:3], op=ALU.mult)
                    nc.vector.tensor_tensor(bund[:, :, 5:6], ut[:, :, 0:1],
                                            ut[:, :, 2:3], op=ALU.mult)
                    nc.vector.tensor_copy(bund[:, :, 6:8], ut[:, :, 1:3])
                    nc.vector.tensor_copy(bund[:, :, 8:9], ut[:, :, 0:1])
                    bunds[s] = bund
                nc.vector.tensor_tensor(gt_g[:, 0:ng, :, :], gt_g[:, 0:ng, :, :],
                                        gt_g[:, 0:ng, :, :], op=ALU.mult)
                s1_g = T(gp, [128, GSUP, CH, 1], F32, "s1", bufs=2)
                nc.scalar.activation(s1_g[:, 0:ng, :, :], db_g[:, 0:ng, :, :],
                                     AF.Sin, bias=ct["halfpi"][:],
                                     scale=math.pi / CUTOFF)
                gx_g = T(gp, [128, GSUP, CH, NB], BF16, "gx", bufs=2)
                nc.scalar.activation(gx_g[:, 0:ng, :, :], gt_g[:, 0:ng, :, :],
                                     AF.Exp, scale=-0.5 * (NB / CUTOFF) ** 2)
                for si, s in enumerate(grp):
                    bund = bunds[s]
                    m1 = T(gp, [128, CH, 1], F32, "m1", bufs=2)
                    nc.vector.tensor_scalar(m1[:], d2_g[:, si, :, :],
                                            CUTOFF * CUTOFF, None,
                                            op0=ALU.is_lt)
                    env = T(gp, [128, CH, 1], F32, "env", bufs=2)
                    nc.vector.tensor_scalar(env[:], s1_g[:, si, :, :], 0.5, 0.5,
                                            op0=ALU.mult, op1=ALU.add)
                    nc.vector.tensor_tensor(env[:], env[:], m1[:], op=ALU.mult)
                    nc.vector.tensor_tensor(
                        bund[:, :, 9:17], gx_g[:, si, :, :],
                        env[:].broadcast_to([128, CH, NB]), op=ALU.mult)
                    bT = T(gp, [32, SUPER], BF16, "bT", bufs=GB)
                    for g4 in range(CH // 4):
                        tp_ps = T(pp, [32, 4, 128], BF16, "ps_tp", bufs=1)
                        for c in range(4):
                            nc.tensor.transpose(tp_ps[:, c, :],
                                                bund[:, g4 * 4 + c, :],
                                                ct["ident"][:])
                        nc.scalar.activation(
                            bT[0:17, g4 * TILE:(g4 + 1) * TILE],
                            tp_ps[0:17, :, :].rearrange("p a b -> p (a b)"),
                            AF.Copy)
                    bTs[s] = bT

                # ---------- per-pair MLP ----------
                pairs = [(s, q) for s in grp for q in range(2)]
                efs = {}
                EFB = 4 * GSUP + 2
                for l in range(NL):
                    efs_prev = dict(efs)
                    vars_ = {}
                    for pi, (s, q) in enumerate(pairs):
                        S, bT = Ss[s], bTs[s]
                        hs = T(fp, [128, 2, TILE], BF16, "hs", bufs=4)
                        for j in range(2):
                            t = 2 * q + j
                            sl = slice(t * TILE, (t + 1) * TILE)
                            isl = slice((t % 2) * TILE, (t % 2) * TILE + TILE)
                            h_ps = T(pp, [128, TILE], F32, "ps_h", bufs=3)
                            nc.tensor.matmul(h_ps[:], ct[f"combo{l}"][:],
                                             bT[0:17, sl], start=True,
                                             stop=False, skip_group_check=True)
                            if l == 0:
                                nc.tensor.matmul(h_ps[0:64, :], ct["I64"][:],
                                                 S[0:64, t, 0, :],
                                                 start=False, stop=True,
                                                 skip_group_check=True)
                            else:
                                if l == 1:
                                    nc.tensor.matmul(h_ps[0:64, :],
                                                     ct["I64b"][64:128, :],
                                                     S[64:128, t, 0, :],
                                                     start=False, stop=False,
                                                     skip_group_check=True)
                                else:
                                    nc.tensor.matmul(h_ps[0:64, :],
                                                     ct["I64"][:],
                                                     S[0:64, t, 2, :],
                                                     start=False, stop=False,
                                                     skip_group_check=True)
                                nc.tensor.matmul(h_ps[0:64, :],
                                                 ct[f"w1a{l}"][:],
                                                 efs_prev[(s, q)][:, j, :],
                                                 start=False, stop=True,
                                                 skip_group_check=True)
                            nc.scalar.activation(hs[:, j, :], h_ps[:], AF.Silu)
                        h2_ps = T(pp, [128, TILE], F32, "ps_one", bufs=1)
                        nc.tensor.matmul(h2_ps[0:64, :], ct[f"w2{l}"][:],
                                         hs[0:64, 0, :], start=True, stop=True,
                                         skip_group_check=True)
                        nc.tensor.matmul(h2_ps[64:128, :], ct[f"w2{l}"][:],
                                         hs[0:64, 1, :], start=True, stop=True,
                                         skip_group_check=True)
                        h2s = T(fp, [128, TILE], BF16, "h2s", bufs=4)
                        nc.scalar.activation(h2s[:], h2_ps[:], AF.Silu)
                        ef_new = T(fp, [128, 2, TILE], BF16, "ef", bufs=EFB)
                        efs[(s, q)] = ef_new
                        var_bc = T(fp, [128, 2, TILE], BF16, "var_bc",
                                   bufs=2 * GSUP + 1)
                        sq2 = T(fp, [128, 2, TILE], BF16, "sq", bufs=3)
                        for j in range(2):
                            t = 2 * q + j
                            sl = slice(t * TILE, (t + 1) * TILE)
                            isl = slice((t % 2) * TILE, (t % 2) * TILE + TILE)
                            xc_ps = T(pp, [128, TILE], F32, "ps_xc", bufs=2)
                            if j == 0:
                                nc.tensor.matmul(xc_ps[:], ct[f"w3C{l}"][:],
                                                 h2s[0:64, :], start=True,
                                                 stop=False)
                            else:
                                nc.tensor.matmul(xc_ps[:],
                                                 ct[f"w3C64_{l}"][64:128, :],
                                                 h2s[64:128, :], start=True,
                                                 stop=False)
                            if l == 0:
                                nc.tensor.matmul(xc_ps[:], ct["CmGeo"][:],
                                                 bT[0:17, sl], start=False,
                                                 stop=False)
                                nc.tensor.matmul(xc_ps[:], ct["ident"][:],
                                                 S[:, t, 1, :],
                                                 start=False, stop=True)
                            else:
                                nc.tensor.matmul(xc_ps[:], ct["CmatX"][:],
                                                 efs_prev[(s, q)][:, j, :],
                                                 start=False, stop=True)
                            wgt_ps = T(pp, [128, TILE], F32, "ps_wgt", bufs=1)
                            nc.tensor.matmul(wgt_ps[:],
                                             ct[f"tp2g64_{l}"][64:128, :],
                                             hs[64:128, j, :], start=True,
                                             stop=True)
                            xcs = T(fp, [128, TILE], BF16, "xcs", bufs=4)
                            nc.vector.tensor_copy(xcs[:], xc_ps[:])
                            nc.vector.tensor_tensor(sq2[:, j, :], xcs[:],
                                                    xcs[:], op=ALU.mult)
                            nc.vector.tensor_tensor(ef_new[:, j, :], xcs[:],
                                                    wgt_ps[:], op=ALU.mult)
                        ai = nc.gpsimd.partition_all_reduce(
                            var_bc[:], sq2[:], 128, bisa.ReduceOp.add)
                        vars_[(s, q)] = var_bc
                    prev_ri = None
                    for (s, q) in pairs:  # ACT: Rsqrt cluster; DVE: ef*=rstd
                        var_bc = vars_[(s, q)]
                        ri = _act_raw(nc, var_bc[:], var_bc[:], AF.Rsqrt,
                                      bias=LN_EPS, scale=1.0 / DD)
                        adh(ri, ai, "batch rsqrt")
                        if prev_ri is not None:
                            adh(ri, prev_ri, "chain rsqrt")
                        prev_ri = ri
                        ef = efs[(s, q)]
                        nc.vector.tensor_tensor(ef[:], ef[:], var_bc[:],
                                                op=ALU.mult)

                # ---------- head ----------
                for s in grp:
                    p2_ps = T(pp, [128, TILE], F32, "ps_wgt", bufs=1)
                    for q in range(2):
                        ef = efs[(s, q)]
                        p1_ps = T(pp, [128, TILE], F32, "ps_xc", bufs=2)
                        for j in range(2):
                            nc.tensor.matmul(p1_ps[64 * j:64 * j + 64, :],
                                             ct["hw1"][:], ef[:, j, :],
                                             start=True, stop=True,
                                             skip_group_check=True)
                        p1s = T(fp, [128, TILE], BF16, "p1s", bufs=2)
                        nc.scalar.activation(p1s[:], p1_ps[:], AF.Silu)
                        nc.tensor.matmul(p2_ps[64 * q:64 * q + 64, :],
                                         ct["hw2st"][:], p1s[:], start=True,
                                         stop=True, skip_group_check=True)
                    p2s = T(fp, [128, TILE], BF16, "p2s", bufs=2)
                    nc.scalar.activation(p2s[:], p2_ps[:], AF.Silu)
                    if s == NS - 1 and NEC_VALID < TILE:
                        nc.vector.memset(p2s[96:128, NEC_VALID:TILE], 0.0)
                    pe_ps = T(pp, [1, TILE], F32, "ps_wgt", bufs=1)
                    nc.tensor.matmul(pe_ps[:], ct["hw3cat"][:], p2s[:],
                                     start=True, stop=True,
                                     skip_group_check=True)
                    nc.vector.tensor_tensor(acc_sb[:], acc_sb[:], pe_ps[:],
                                            op=ALU.add)

            # ---------- finals ----------
            red1 = T(cp, [1, 1], F32, "red1")
            nc.vector.tensor_reduce(red1[:], acc_sb[:],
                                    axis=mybir.AxisListType.X, op=ALU.add)
            tot = T(cp, [1, 1], F32, "tot")
            nc.vector.tensor_tensor(tot[:], red1[:], ct["ebias"][:], op=ALU.add)
            nc.sync.dma_start(out_d.ap()[:], tot[:])

    return d_in


def _prep_host(inputs):
    an = np.asarray(inputs["atomic_numbers"]).astype(np.int64)
    pos = np.asarray(inputs["pos"], np.float32)
    ei = np.asarray(inputs["edge_index"]).astype(np.int64)
    E = ei.shape[1]
    assert E % N_CORES == 0
    EC = E // N_CORES
    EC_PAD = ((EC + SUPER - 1) // SUPER) * SUPER

    f32 = lambda x: np.ascontiguousarray(x, np.float32)
    bf = lambda x: np.ascontiguousarray(
        np.asarray(x, np.float32).astype(ml_dtypes.bfloat16))

    Ssh = _sh_mix_matrix()
    Cm = (np.eye(DD) - np.ones((DD, DD)) / DD).astype(np.float32)

    emb = f32(inputs["node_emb"])
    w_init = f32(inputs["w_init"])
    w1 = f32(inputs["w1"]); w2 = f32(inputs["w2"]); w3 = f32(inputs["w3"])
    tpw1 = f32(inputs["tpw1"]); tpw2 = f32(inputs["tpw2"])
    g_ln = f32(inputs["ln_g"])

    for k in ("b1", "b2", "b3", "ln_b", "tpb1", "tpb2", "hb1", "hb2"):
        assert not np.any(np.asarray(inputs[k])), f"{k} nonzero unsupported"

    # per-(z_r, z_c) pair contributions
    A_r = emb @ w_init[0:H]            # [100,128]
    A_c = emb @ w_init[H:2 * H]
    A_pair = A_r[:, None, :] + A_c[None, :, :]          # [100,100,128]
    A_pair = A_pair + f32(inputs["b_init"])[None, None, :]
    c1 = []
    for l in range(NL):
        P = emb @ w1[l][DD:DD + H]
        Q = emb @ w1[l][DD + H:DD + 2 * H]
        c1.append(P[:, None, :] + Q[None, :, :])        # [100,100,64]
    w1a0 = w1[0][0:DD]                                  # [128,64]
    D0 = A_pair @ w1a0 + c1[0]                          # [100,100,64]
    CA = A_pair @ Cm                                    # [100,100,128]
    spt = np.zeros((100, 100, 384), np.float32)
    spt[:, :, 0:64] = D0
    spt[:, :, 64:128] = c1[1]
    spt[:, :, 128:256] = CA
    spt[:, :, 256:320] = c1[2]
    spt = bf(spt.reshape(10000, 384))

    # geometry weight rows: bT features = [ab9(9); rbf(8)]
    Wgeo = np.zeros((17, DD), np.float32)
    Wgeo[0:9] = Ssh.T @ w_init[2 * H + NB:]
    Wgeo[9:17] = w_init[2 * H:2 * H + NB]

    consts = {
        "spt": spt,
        "I64": bf(np.eye(64)),
        "ones1": bf(np.ones((128, 1))),
        "ident": bf(np.eye(128)),
        "hw1": bf(inputs["hw1"]),
        "halfpi": f32(np.full((128, 1), math.pi / 2)),
        "cents8": f32(np.tile(CENTS.astype(np.float32).reshape(1, 1, NB),
                              (128, 1, 1))),
        "CmGeo": bf(Wgeo @ Cm),
        "CmatX": bf(Cm),
    }
    I64b = np.zeros((128, 64), np.float32)
    I64b[64:128] = np.eye(64)
    consts["I64b"] = bf(I64b)
    for l in range(NL):
        combo = np.zeros((17, 128), np.float32)
        combo[9:17, 0:64] = w1[l][DD + 2 * H:]      # rbf -> h1
        combo[0:9, 64:128] = Ssh.T @ tpw1[l]        # ab9 -> tp1 pre
        if l == 0:
            combo[:, 0:64] += Wgeo @ w1a0           # folded ef0 geometry
        consts[f"combo{l}"] = bf(combo)
        consts[f"w2{l}"] = bf(w2[l])
        w3C = w3[l] @ Cm
        consts[f"w3C{l}"] = bf(w3C)
        w3C64 = np.zeros((128, DD), np.float32)
        w3C64[64:128] = w3C
        consts[f"w3C64_{l}"] = bf(w3C64)
        tp2g = tpw2[l] * g_ln[l][None, :]
        tp2g64 = np.zeros((128, DD), np.float32)
        tp2g64[64:128] = tp2g
        consts[f"tp2g64_{l}"] = bf(tp2g64)
    for l in range(1, NL):
        consts[f"w1a{l}"] = bf(w1[l][0:DD])
    hw2 = f32(inputs["hw2"])
    hw2st = np.zeros((128, 64), np.float32)
    hw2st[0:64, 0:32] = hw2
    hw2st[64:128, 32:64] = hw2
    consts["hw2st"] = bf(hw2st)
    hw3 = f32(inputs["hw3"]).reshape(32)
    hw3cat = np.zeros((128, 1), np.float32)
    for j in range(4):
        hw3cat[32 * j:32 * j + 32, 0] = hw3
    consts["hw3cat"] = bf(hw3cat)

    # pos pair table (256B rows — SWDGE gather minimum granularity)
    NPAIRS = (NN + 1) // 2
    ptab = np.zeros((NPAIRS, 64), np.float32)
    ptab[:, 0:3] = pos[0::2]
    n_odd = NN // 2
    ptab[:n_odd, 32:35] = pos[1::2]
    consts["ptab"] = ptab

    ae = np.asarray(inputs["atomic_e"], np.float32).reshape(-1)
    ae_total = float(np.float64(ae[an].astype(np.float64).sum()))
    hb3 = float(np.asarray(inputs["hb3"]).reshape(-1)[0])

    z_r = an[ei[0]]
    z_c = an[ei[1]]
    sp_idx = (z_r * 100 + z_c).astype(np.int16)

    in_maps = []
    for c in range(N_CORES):
        im = dict(consts)
        e0 = c * EC
        idx = ei[:, e0:e0 + EC]
        spi = sp_idx[e0:e0 + EC]
        pad = EC_PAD - EC
        if pad:
            idx = np.concatenate([idx, np.zeros((2, pad), np.int64)], 1)
            spi = np.concatenate([spi, np.zeros(pad, np.int16)])
        for side, nm_i, nm_p in ((0, "idxr", "parr"), (1, "idxc", "parc")):
            nodes = idx[side]
            im[nm_i] = _wrap_idx((nodes // 2).astype(np.int16), 1024)
            par = (nodes & 1).astype(np.uint8)
            pe = par.reshape(-1, 128).T
            im[nm_p] = np.ascontiguousarray(
                np.repeat(pe[:, :, None], 3, axis=2))
        im["idxs"] = _wrap_idx(spi, TILE)
        im["ebias"] = f32([[EC * hb3 + (ae_total if c == 0 else 0.0)]])
        in_maps.append(im)

    nz = {"halfpi": True}
    return in_maps, {"EC_PAD": EC_PAD, "NAT_PAD": 0, "nz": nz}


def _make_nc():
    return bacc.Bacc("TRN2", target_bir_lowering=False, debug=False,
                     num_devices=N_CORES, num_swdge_queues=2)


def prepare(inputs):
    t0 = time.time()
    in_maps, meta = _prep_host(inputs)
    t1 = time.time()
    nc = _make_nc()
    _build(nc, meta["EC_PAD"], meta["NAT_PAD"], meta["nz"])
    t2 = time.time()
    nc.compile()
    t3 = time.time()
    if os.environ.get("KERNEL_VERBOSE"):
        print(f"[kernel] prep {t1-t0:.1f}s build {t2-t1:.1f}s "
              f"bir-compile {t3-t2:.1f}s", flush=True)
    return nc, in_maps


def kernel(**inputs) -> np.ndarray:
    nc, in_maps = prepare(inputs)
    res = run_bass_kernel_spmd(nc, in_maps, core_ids=list(range(N_CORES)))
    total = np.zeros((1, 1), np.float32)
    for r in res.results:
        total += r["out"]
    return total
